# revision 56
# baseline (speedup 1.0000x reference)
"""Trainium2 Bass kernel for the CoAttnLayer problem.

Computes, per example b (B=16, all dims 1024):
    P   = E @ W^T                      (proj)
    S   = P @ Src^T                    (sim, (e, l))
    edit_w  = softmax_l(S + srcmask)   -> edit_ctx = edit_w @ Src
    src_w   = softmax_e(S + editmask)  -> src_ctx  = src_w^T @ E

Sharding: data-parallel over batch, 2 examples per core on 8 cores; W^T
replicated (loaded into SBUF once per core). Logit-path matmuls (proj,
sim) run in float32r; the softmax-weight context matmuls run in bf16
(weights in [0,1] and encodings ~N(0,1), so bf16 rounding contributes
~2-4e-3 relative error, well inside the 2e-2 gate).

Schedule highlights (310.5us -> ~299.1us over two sessions):
  - W^T loads once per core; ~5us of throwaway identity matmuls
    pre-warm the PE's HAM clock gate until the first wt/et chunks land.
  - Phase 1 (PT = W @ E^T): for b=0, k-outer over 3 column groups so
    compute starts on the first DMA'd k-slices; b0's srct/srcn
    prefetches are deferred into the group loop ON GPSIMD (posting from
    scalar/sync stalls compute behind DMA ring-full waits) so wt+et get
    the startup bandwidth. et posts one full 4KB k-row per descriptor.
  - Phase 2 per mb: sim(mb+1) pipelined ahead; the eexp transposes
    read the UNMASKED exp (srcn rows for masked l are zeroed on the
    host instead), so the Z mask+row-sum — one fused DVE
    scalar_tensor_tensor — runs off the transpose critical path; each
    block's row-max is hoisted into the previous block (computed as
    soon as sim(mb) stops). Output evictions deferred one iteration.
  - Phase 2 -> 3 boundary: phase-3 row-maxes for rows 0/1 computed as
    split halves (left at mb5, right + min-combine at mb7), exp3(0/1)
    issue inside mb7, and the first phase-3 transpose batch rides
    INSIDE mb7's ctx matmul run (12/4 split).
  - Phase 3: transposes for lb+1 issue in the MIDDLE of ctx(lb)'s run
    so their psum->SBUF copy overlaps its tail (no DVE wait at the
    iteration boundary); exp 1 ahead, reduce_max 2 ahead; evictions
    flushed ONE per iteration — each ctx3 completes within its own
    iteration, and the every-iteration flush keeps the eexp-tag slot
    rotation strictly alternating ex3/stage, so no exp3 ever lands on
    a slot whose transpose reads haven't run (this alone was ~5us:
    deferral-2 broke the alternation at lb1 and serialized exp3s
    behind same-slot transposes). ctx3(0)/odd lbs borrow "sim" psum
    slots; prefetch_main(b+1) posts at mb4 so the gpsimd ring spreads
    the next example's 12MB against phase 3's output evictions.
  - PE transposes: S^T logit tiles in f32r (~fp22 truncation, measured
    ~4e-4 output contribution), exp-weight tiles in bf16.
  - Outputs are scaled into bf16 SBUF stages and stored as bf16 (host
    converts to f32), halving output DMA.

Masking (mask is additive -1e9, like the reference's -inf up to fp32
underflow of the resulting exp):
  - src mask (over l) for the edit path: row-max taken unmasked
    (softmax shift-invariance); masked columns contribute zero to the
    ctx matmul through the host-zeroed srcn rows and are excluded from
    Z by the fused (exp * s01) row-sum.
  - edit mask (over e) for the src path: added as a per-partition bias
    before the S^T transposes, so exp() yields exact zeros for masked
    rows and Z comes from the activation accumulate output.
"""

import os
import sys

for _p in ("/opt/trn_rl_repo",):
    if os.path.isdir(_p) and _p not in sys.path:
        sys.path.insert(0, _p)

import numpy as np
import ml_dtypes

import concourse.bass as bass
import concourse.tile as tile
from concourse import mybir
from concourse.bass import ts
from concourse.masks import make_identity
from concourse.bass_utils import run_bass_kernel_spmd

B = 16
L = 1024          # LE = LS = DE = DS = 1024
NCORES = 8
PER_CORE = B // NCORES
P = 128           # partitions
KB = L // P       # 8 k-tiles
N2 = 2            # two 512-wide n-tiles
F32 = mybir.dt.float32
F32R = mybir.dt.float32r
BF16 = mybir.dt.bfloat16
AX = mybir.AxisListType.X
EXP = mybir.ActivationFunctionType.Exp
MUL = mybir.AluOpType.mult
ADD = mybir.AluOpType.add
NEG = -1.0e9
NPBF16 = ml_dtypes.bfloat16

# phase-1 column groups (start sb, width) and the psum tag for each member
G3 = [(0, 3), (3, 3), (6, 2)]


def build_nc(n_examples=PER_CORE, fence=True):
    nc = bass.Bass("TRN2", target_bir_lowering=False)

    et_d = nc.dram_tensor("et", (n_examples, L, L), F32, kind="ExternalInput")     # E^T  (d, e)
    srct_d = nc.dram_tensor("srct", (n_examples, L, L), F32, kind="ExternalInput")  # Src^T (s, l)
    srcn_d = nc.dram_tensor("srcn", (n_examples, L, L), BF16, kind="ExternalInput")  # Src natural (l, s)
    en_d = nc.dram_tensor("en", (n_examples, L, L), BF16, kind="ExternalInput")     # E natural (e, d)
    wt_d = nc.dram_tensor("wt", (L, L), F32, kind="ExternalInput")                  # W^T (d, s)
    emcol_d = nc.dram_tensor("emcol", (n_examples, P, KB), F32, kind="ExternalInput")  # additive edit mask [p, eb]
    s01r_d = nc.dram_tensor("s01r", (n_examples, L), BF16, kind="ExternalInput")    # src validity 0/1 row
    oe_d = nc.dram_tensor("oe", (n_examples, L, L), BF16, kind="ExternalOutput")    # edit_ctx
    os_d = nc.dram_tensor("osr", (n_examples, L, L), BF16, kind="ExternalOutput")   # src_ctx

    with tile.TileContext(nc) as tc:
        with (
            tc.tile_pool(name="persist", bufs=1) as persist,
            tc.tile_pool(name="big", bufs=4) as big,
            tc.tile_pool(name="bigh", bufs=2) as bigh,
            tc.tile_pool(name="blk", bufs=2) as blk,
            tc.tile_pool(name="expt", bufs=2) as expt,
            tc.tile_pool(name="stats", bufs=8) as stats,
            tc.tile_pool(name="small", bufs=2) as small,
            tc.tile_pool(name="ps", bufs=1, space="PSUM") as psum,
        ):
            # ~3us of throwaway matmuls while the first wt/et DMAs stream in:
            # keeps the PE busy through the HAM activity window so the real
            # phase-1 matmuls start at the warm 2.4 GHz clock instead of 1.2.
            # Zero-tile memset issues first so the PE isn't waiting on the
            # identity construction.
            warmz = persist.tile([P, P], BF16, tag="warmz")
            nc.gpsimd.memset(warmz, 0)
            warm = psum.tile([P, P], F32, tag="tr", bufs=2, name="warm")
            for _ in range(52):
                nc.tensor.matmul(warm, warmz, warmz, start=True, stop=True)

            # identity built in bf16 (make_identity can't memset f32r),
            # then cast-copied to f32r for the logit transposes
            identb = persist.tile([P, P], BF16, tag="identb")
            make_identity(nc, identb)
            identr = persist.tile([P, P], F32R, tag="identr")
            nc.vector.tensor_copy(identr, identb)

            # W^T resident for the whole kernel; column-group chunks issued
            # k-ascending so group 0's accumulation can start immediately.
            wt_sb = persist.tile([P, KB, L], F32R, tag="wt")
            for s0, w in G3:
                for k in range(KB):
                    nc.sync.dma_start(
                        out=wt_sb[:, k, s0 * P : (s0 + w) * P],
                        in_=wt_d[k * P : (k + 1) * P, s0 * P : (s0 + w) * P].bitcast(F32R),
                    )

            def post_et(b, et_sb, eng=None):
                # one full-row post per k-slice: halves the post count so the
                # queue's ~650ns/post serialization doesn't pace the startup
                eng = eng or nc.gpsimd
                for k in range(KB):
                    eng.dma_start(
                        out=et_sb[:, k, :],
                        in_=et_d[b, k * P : (k + 1) * P, :].bitcast(F32R),
                    )

            def post_kslices(dram, sb_tile, cast=None, eng=None):
                eng = eng or nc.gpsimd
                for k in range(KB):
                    src = dram[k * P : (k + 1) * P, :]
                    if cast is not None:
                        src = src.bitcast(cast)
                    eng.dma_start(out=sb_tile[:, k, :], in_=src)

            # per-example SBUF tiles, allocated lazily in rotation order
            et_t = [None] * n_examples
            srct_t = [None] * n_examples
            srcn_t = [None] * n_examples
            en_t = [None] * n_examples

            def prefetch_srct(b, eng=None):
                srct_t[b] = big.tile([P, KB, L], F32R, tag="big", name=f"srct_{b}")
                post_kslices(srct_d[b], srct_t[b], cast=F32R, eng=eng)

            def prefetch_srcn(b, eng=None):
                srcn_t[b] = bigh.tile([P, KB, L], BF16, tag="bigh", name=f"srcn_{b}")
                post_kslices(srcn_d[b], srcn_t[b], eng=eng)

            def prefetch_main(b):
                """et/srct/srcn for example b: allocate + post DMAs (gpsimd)."""
                et_t[b] = big.tile([P, KB, L], F32R, tag="big", name=f"et_{b}")
                post_et(b, et_t[b], eng=nc.gpsimd)
                prefetch_srct(b)
                prefetch_srcn(b)

            def prefetch_en(b):
                en_t[b] = bigh.tile([P, KB, L], BF16, tag="bigh", name=f"en_{b}")
                post_kslices(en_d[b], en_t[b])

            # b0: only et posts up front — srct/srcn are deferred into the
            # phase-1 column-group loop so wt+et get the full HBM bandwidth
            # while phase 1 is DMA-paced (srct isn't read until ~55us,
            # srcn until ~64us).
            et_t[0] = big.tile([P, KB, L], F32R, tag="big", name="et_0")
            post_et(0, et_t[0], eng=nc.gpsimd)

            for b in range(n_examples):
                et_sb = et_t[b]

                emcol_sb = small.tile([P, KB], F32, tag="emcol", name=f"emc_{b}")
                nc.gpsimd.dma_start(out=emcol_sb, in_=emcol_d[b])
                s01rep = small.tile([P, L], BF16, tag="s01rep", bufs=1, name=f"s01_{b}")
                s01_src = bass.AP(
                    tensor=s01r_d[b].tensor,
                    offset=s01r_d[b].offset,
                    ap=[[0, P]] + list(s01r_d[b].ap),
                )
                nc.gpsimd.dma_start(out=s01rep, in_=s01_src)

                # ---- Phase 1: PT = W @ E^T laid out (s, e).
                # b==0: k-outer inside each column group so compute starts on
                # the first DMA'd k-slices (wt/et are still streaming in).
                # b>=1 (everything SBUF-resident): sb-outer, so the first
                # sb's matmuls cover the eviction latency of the previous
                # example's last ctx psum tiles (which share these slots). ----
                pt_sb = big.tile([P, KB, L], F32R, tag="big", name=f"pt_{b}")
                cp_eng = [
                    lambda o, i: nc.scalar.copy(o, i),
                    lambda o, i: nc.vector.tensor_copy(o, i),
                ]
                for gi, (s0, w) in enumerate(G3):
                    tags = (
                        ["sim", "sim", "cps"] if b == 0 else ["cps", "sim", "sim"]
                    )[: w] if w == 3 else ["sim", "sim"]
                    pss = [
                        psum.tile(
                            [P, L], F32, tag=tags[j],
                            bufs=(2 if tags[j] == "sim" else 1),
                            name=f"p1_{b}_{s0}_{j}",
                        )
                        for j in range(w)
                    ]
                    loop = (
                        [(k, j) for k in range(KB) for j in range(w)]
                        if b == 0
                        else [(k, j) for j in range(w) for k in range(KB)]
                    )
                    for k, j in loop:
                        sb = s0 + j
                        for n in range(N2):
                            nc.tensor.matmul(
                                pss[j][:, ts(n, 512)],
                                wt_sb[:, k, ts(sb, P)],
                                et_sb[:, k, ts(n, 512)],
                                start=(k == 0),
                                stop=(k == KB - 1),
                            )
                        if b != 0 and k == KB - 1:
                            cp_eng[j % 2](pt_sb[:, sb, :], pss[j])
                    if b == 0:
                        for j in range(w):
                            cp_eng[j % 2](pt_sb[:, s0 + j, :], pss[j])
                        # deferred src-side prefetches on gpsimd (idle during
                        # phase 1, and ring-full waits can't block compute
                        # there — posting from scalar/sync stalls the phase-1
                        # psum copies behind the congested DMA rings)
                        if gi == 0:
                            prefetch_srct(0)
                        elif gi == 1:
                            prefetch_srcn(0)

                if b == 0:
                    prefetch_en(0)
                en_sb = en_t[b]
                srct_sb = srct_t[b]
                srcn_sb = srcn_t[b]

                # S^T + editmask accumulates here for the src path
                st_sb = big.tile([P, KB, L], F32, tag="big", name=f"st_{b}")

                # ---- phase-3 state + issue helpers (the phase-2 tail
                # pre-issues the src-path prologue so the first phase-3
                # transposes/ctx start with no DVE/ACT latency exposed) ----
                negmax2 = {}
                negmax3 = {}
                nm3h = {}
                exps3 = {}
                izl3 = {}
                sw3 = {}

                def issue_max3(lb):
                    nm = stats.tile([P, 1], F32, tag="negmax", name=f"nml_{b}_{lb}")
                    nc.vector.reduce_max(nm, st_sb[:, lb, :], axis=AX, negate=True)
                    negmax3[lb] = nm

                def issue_max3_half(lb, half):
                    nm = stats.tile([P, 1], F32, tag="negmax", name=f"nmh_{b}_{lb}_{half}")
                    nc.vector.reduce_max(
                        nm, st_sb[:, lb, ts(half, 512)], axis=AX, negate=True
                    )
                    return nm

                def issue_exp3(lb):
                    zl = stats.tile([P, 1], F32, tag="zl", name=f"zl_{b}_{lb}")
                    # shares the eexp rotation; the pend-flush-before-alloc
                    # ordering in phase 2 keeps each ex3 landing on the slot
                    # of an already-evicted stage tile, never on the eexp
                    # the fused Z pass still reads
                    ex = blk.tile([P, L], BF16, tag="eexp", bufs=2, name=f"ex_{b}_{lb}")
                    nc.scalar.activation(
                        ex, st_sb[:, lb, :], EXP, bias=negmax3[lb], accum_out=zl
                    )
                    izl = stats.tile([P, 1], F32, tag="iz", name=f"izl_{b}_{lb}")
                    nc.vector.reciprocal(izl, zl)
                    exps3[lb] = ex
                    izl3[lb] = izl

                def issue_tr3(lb):
                    sw = expt.tile([P, KB, P], BF16, tag="expt", name=f"sw_{b}_{lb}")
                    for g in range(2):
                        tr = psum.tile(
                            [P, 4, P], BF16, tag="tr", bufs=2, name=f"trs_{b}_{lb}_{g}"
                        )
                        for i in range(4):
                            nc.tensor.transpose(
                                tr[:, i, :], exps3[lb][:, ts(4 * g + i, P)], identb
                            )
                        nc.vector.tensor_copy(sw[:, 4 * g : 4 * g + 4, :], tr)
                    sw3[lb] = sw

                def sim_block(mb, ps):
                    # last block runs n-outer so its first half finishes
                    # ~1.8us early and the boundary softmax can start on it
                    loop = (
                        [(k, n) for n in range(N2) for k in range(KB)]
                        if mb == KB - 1
                        else [(k, n) for k in range(KB) for n in range(N2)]
                    )
                    for k, n in loop:
                        nc.tensor.matmul(
                            ps[:, ts(n, 512)],
                            pt_sb[:, k, ts(mb, P)],
                            srct_sb[:, k, ts(n, 512)],
                            start=(k == 0),
                            stop=(k == KB - 1),
                        )

                # ---- Phase 2: edit path per e-block; also builds ST.
                # Two levels of software pipelining: sim(mb+1) issues to the
                # PE before the softmax-dependent transposes of mb, and the
                # output-scale epilogue of mb is deferred into iteration mb+1
                # so the ctx-dependent mul never blocks add/exp on the ACT
                # FIFO. ----
                def oe_epilogue(mb, cps, iz):
                    # ACT muls, demoted in scheduler priority: a ctx-dependent
                    # mul ordered ahead of the next block's exp on the ACT
                    # FIFO would stall the PE transposes (DVE muls instead
                    # slow every matmul ~40ns via PSUM port contention)
                    oe_stage = blk.tile([P, L], BF16, tag="eexp", bufs=2, name=f"oes_{b}_{mb}")
                    with tc.high_priority(offset=-150):
                        for n in range(N2):
                            nc.scalar.mul(oe_stage[:, ts(n, 512)], cps[:, ts(n, 512)], mul=iz)
                            nc.sync.dma_start(
                                out=oe_d[b, mb * P : (mb + 1) * P, ts(n, 512)],
                                in_=oe_stage[:, ts(n, 512)],
                            )

                ps_cur = psum.tile([P, L], F32, tag="sim", bufs=2, name=f"p2_{b}_0")
                sim_block(0, ps_cur)
                pend = None
                for mb in range(KB):
                    if mb + 1 < KB:
                        ps_next = psum.tile(
                            [P, L], F32, tag="sim", bufs=2, name=f"p2_{b}_{mb + 1}"
                        )
                        sim_block(mb + 1, ps_next)
                    else:
                        ps_next = None
                    # raw sim + edit mask (per-partition bias) for the src path
                    smask = blk.tile([P, L], F32R, tag="smask", bufs=1, name=f"sm_{b}_{mb}")
                    if mb == KB - 1:
                        for n in range(N2):
                            nc.scalar.add(
                                smask[:, ts(n, 512)], ps_cur[:, ts(n, 512)],
                                add=emcol_sb[:, mb : mb + 1],
                            )
                    else:
                        nc.scalar.add(smask, ps_cur, add=emcol_sb[:, mb : mb + 1])
                    if mb in negmax2:
                        negmax = negmax2[mb]
                    else:
                        negmax = stats.tile([P, 1], F32, tag="negmax", name=f"nm_{b}_{mb}")
                        nc.vector.reduce_max(negmax, ps_cur, axis=AX, negate=True)
                    eexp = blk.tile([P, L], BF16, tag="eexp", bufs=2, name=f"ee_{b}_{mb}")
                    nc.scalar.activation(eexp, ps_cur, EXP, bias=negmax)
                    if pend is not None:
                        pend[0](*pend[1:])
                    # masked sim -> ST columns (f32 transposes; ST copies on
                    # the ACT engine so DVE only carries the bf16 copies)
                    def smask_section(g):
                        tr = psum.tile([P, 4, P], F32R, tag="tr", bufs=2, name=f"trm_{b}_{mb}_{g}")
                        for i in range(4):
                            nc.tensor.transpose(tr[:, i, :], smask[:, ts(4 * g + i, P)], identr)
                        nc.scalar.copy(st_sb[:, 4 * g : 4 * g + 4, ts(mb, P)], tr)

                    # exp(S) -> (l, e-block) bf16 for the edit ctx matmul.
                    # The transposes read the UNMASKED exp (srcn rows for
                    # masked l are zeroed on the host, so masked columns
                    # contribute nothing to the ctx matmul regardless).
                    eexpT = expt.tile([P, KB, P], BF16, tag="expt", name=f"eeT_{b}_{mb}")

                    def eexpT_section(g):
                        tr = psum.tile([P, 4, P], BF16, tag="tr", bufs=2, name=f"tre_{b}_{mb}_{g}")
                        for i in range(4):
                            nc.tensor.transpose(tr[:, i, :], eexp[:, ts(4 * g + i, P)], identb)
                        nc.vector.tensor_copy(eexpT[:, 4 * g : 4 * g + 4, :], tr)

                    smask_section(0)
                    smask_section(1)
                    eexpT_section(0)
                    eexpT_section(1)
                    # Z_e: fused mask-multiply + row-sum in ONE DVE pass
                    # (walrus only lowers TensorScalarPtr on the DVE)
                    zps = stats.tile([P, 1], F32, tag="zl", name=f"ze_{b}_{mb}")
                    nc.vector.scalar_tensor_tensor(
                        out=eexp, in0=eexp, scalar=1.0, in1=s01rep,
                        op0=mybir.AluOpType.bypass, op1=MUL, accum_out=zps,
                    )
                    iz = stats.tile([P, 1], F32, tag="iz", name=f"iz_{b}_{mb}")
                    nc.vector.reciprocal(iz, zps)
                    if mb + 1 < KB:
                        # next block's row-max hoisted here: the DVE computes
                        # it as soon as sim(mb+1) stops (mid-block), so the
                        # next block's exp — and at mb7 the whole phase-3
                        # prologue chain — isn't headed by a 1.2us reduce
                        nm_n = stats.tile(
                            [P, 1], F32, tag="negmax", name=f"nm_{b}_{mb + 1}"
                        )
                        nc.vector.reduce_max(nm_n, ps_next, axis=AX, negate=True)
                        negmax2[mb + 1] = nm_n
                    if mb == KB - 3:
                        # left-half row maxes of ST rows 0/1 (their columns
                        # for mb 0..3 are final); right halves + combine
                        # happen at mb7 so the phase-3 prologue exps can
                        # issue with almost no reduce latency exposed
                        nm3h[0] = issue_max3_half(0, 0)
                        nm3h[1] = issue_max3_half(1, 0)
                    if mb == KB - 1:
                        for lb3 in range(2):
                            nmb = issue_max3_half(lb3, 1)
                            nmc = stats.tile(
                                [P, 1], F32, tag="negmax", name=f"nml_{b}_{lb3}"
                            )
                            nc.vector.tensor_tensor(
                                nmc, nm3h[lb3], nmb, mybir.AluOpType.min
                            )
                            negmax3[lb3] = nmc
                        issue_exp3(0)
                        issue_exp3(1)
                    cps = psum.tile([P, L], F32, tag="cps", bufs=1, name=f"ec_{b}_{mb}")
                    cloop = [(k, n) for k in range(KB) for n in range(N2)]
                    if mb == KB - 1:
                        # split the last ctx run so the first phase-3
                        # transpose batch rides inside it: its sw copy then
                        # overlaps the run's tail instead of stalling ctx3(0)
                        for k, n in cloop[:12]:
                            nc.tensor.matmul(
                                cps[:, ts(n, 512)], eexpT[:, k, :],
                                srcn_sb[:, k, ts(n, 512)],
                                start=(k == 0), stop=(k == KB - 1),
                            )
                        issue_tr3(0)
                        for k, n in cloop[12:]:
                            nc.tensor.matmul(
                                cps[:, ts(n, 512)], eexpT[:, k, :],
                                srcn_sb[:, k, ts(n, 512)],
                                start=(k == 0), stop=(k == KB - 1),
                            )
                    else:
                        for k, n in cloop:
                            nc.tensor.matmul(
                                cps[:, ts(n, 512)], eexpT[:, k, :],
                                srcn_sb[:, k, ts(n, 512)],
                                start=(k == 0), stop=(k == KB - 1),
                            )
                    pend = (oe_epilogue, mb, cps, iz)
                    if mb == KB - 4 and b + 1 < n_examples:
                        # prefetch early (ring-full waits on gpsimd hold the
                        # posts until the slots' last readers finish): spreads
                        # the next example's 12MB over ~70us instead of
                        # bursting it against phase 3's output evictions
                        prefetch_main(b + 1)
                    ps_cur = ps_next

                # ---- Phase 3: src path per l-block, from ST in SBUF.
                # Same deferred-epilogue pattern; phase 2's last epilogue is
                # issued after phase 3's first exp so it can't block it. ----
                def os_epilogue(lb, cps, izl, on_dve=False):
                    os_stage = blk.tile([P, L], BF16, tag="eexp", bufs=2, name=f"oss_{b}_{lb}")
                    with tc.high_priority(offset=-150):
                        for n in range(N2):
                            if on_dve:
                                nc.vector.tensor_scalar_mul(
                                    os_stage[:, ts(n, 512)], cps[:, ts(n, 512)], izl
                                )
                            else:
                                nc.scalar.mul(
                                    os_stage[:, ts(n, 512)], cps[:, ts(n, 512)], mul=izl
                                )
                            nc.sync.dma_start(
                                out=os_d[b, lb * P : (lb + 1) * P, ts(n, 512)],
                                in_=os_stage[:, ts(n, 512)],
                            )

                # Phase 3 proper. Software pipeline depths: reduce_max 2
                # ahead, exp 1 ahead, transposes issued INSIDE the previous
                # lb's ctx run (so their sw copy overlaps its tail and ctx3
                # never waits on the DVE). Evictions deferred 2 iterations as
                # before. ctx3(0)/ctx3(1) borrow the idle "sim" slots so the
                # first ctx3 isn't gated on phase 2's cps eviction.
                pends = [pend] if pend is not None else []
                pend = None
                for lb in range(KB):
                    if lb + 2 < KB and (lb + 2) not in negmax3:
                        issue_max3(lb + 2)
                    if lb + 1 < KB and (lb + 1) not in exps3:
                        issue_exp3(lb + 1)
                    # lb==0 flushes phase 2's handoff epilogue; afterwards
                    # flush one eviction EVERY iteration (each ctx3 completes
                    # within its own iteration, so the mul's dependency is
                    # resolved by the next lb top). The every-iteration flush
                    # also keeps the eexp-tag rotation strictly alternating
                    # ex3/stage, so no exp3 lands on a slot whose transposes
                    # haven't run yet.
                    if lb == 0:
                        for p in pends:
                            p[0](*p[1:])
                        pends = []
                    elif pends:
                        p = pends.pop(0)
                        p[0](*p[1:])
                    if lb not in sw3:
                        issue_tr3(lb)
                    sw = sw3[lb]
                    if lb != KB - 1:
                        ctag = "sim" if (lb % 2 == 1 or lb == 0) else "cps"
                        cps = psum.tile(
                            [P, L], F32, tag=ctag, bufs=(1 if ctag == "cps" else 2),
                            name=f"sc_{b}_{lb}",
                        )
                    if lb == KB - 1:
                        # last block of every example: the two n-halves
                        # accumulate in two SEPARATE psum tiles (separate
                        # dependency tracking — slices of one tile serialize
                        # on the whole tile), so each half's eviction
                        # mul+store overlaps the other half's matmuls. The
                        # "tr" slots are free here (last transposes ran at
                        # lb6), and the eviction no longer holds a sim/cps
                        # slot — the next example's phase 1 starts ungated.
                        os_fin = blk.tile([P, L], BF16, tag="eexp", bufs=2, name=f"osf_{b}")
                        for n in range(N2):
                            cph = psum.tile(
                                [P, 512], F32, tag="tr", bufs=2, name=f"scf_{b}_{n}"
                            )
                            for k in range(KB):
                                nc.tensor.matmul(
                                    cph, sw[:, k, :], en_sb[:, k, ts(n, 512)],
                                    start=(k == 0), stop=(k == KB - 1),
                                )
                            nc.scalar.mul(
                                os_fin[:, ts(n, 512)], cph, mul=izl3[lb]
                            )
                            nc.sync.dma_start(
                                out=os_d[b, lb * P : (lb + 1) * P, ts(n, 512)],
                                in_=os_fin[:, ts(n, 512)],
                            )
                        for p in pends:
                            p[0](*p[1:])
                        pends = []
                        continue
                    if lb == KB - 1:
                        for k, n in [(k, n) for n in range(N2) for k in range(KB)]:
                            nc.tensor.matmul(
                                cps[:, ts(n, 512)], sw[:, k, :],
                                en_sb[:, k, ts(n, 512)],
                                start=(k == 0), stop=(k == KB - 1),
                            )
                    else:
                        c3loop = [(k, n) for k in range(KB) for n in range(N2)]
                        for k, n in c3loop[:10]:
                            nc.tensor.matmul(
                                cps[:, ts(n, 512)], sw[:, k, :],
                                en_sb[:, k, ts(n, 512)],
                                start=(k == 0), stop=(k == KB - 1),
                            )
                        issue_tr3(lb + 1)
                        for k, n in c3loop[10:]:
                            nc.tensor.matmul(
                                cps[:, ts(n, 512)], sw[:, k, :],
                                en_sb[:, k, ts(n, 512)],
                                start=(k == 0), stop=(k == KB - 1),
                            )
                    pends.append((os_epilogue, lb, cps, izl3[lb]))
                    if lb == KB - 1:
                        # flush all but the newest: their ctx deps are done,
                        # so the next example's phase 1 (or the kernel drain)
                        # isn't gated on evictions that haven't been issued
                        while len(pends) > 1:
                            p = pends.pop(0)
                            p[0](*p[1:])
                    if lb == 0 and b + 1 < n_examples:
                        prefetch_en(b + 1)
                # drain the last epilogue of the example; on the DVE for
                # non-final examples (ACT is clogged with the boundary exps
                # and the next phase 1 waits on this psum slot)
                for p in pends:
                    p[0](*p[1:], on_dve=(b + 1 < n_examples))
                pends = []

    if fence:
        _fence_matmul_waits(nc)
    return nc


def _fence_matmul_waits(nc):
    """walrus can attach at most one sync wait to the LDWEIGHTS half of a
    self-loading fp32/fp32r matmul. Move every multi-wait Matmult's waits
    onto a PE no-op fence inserted right before it (the NX sequencer
    performs waits in issue order, so the fence strictly precedes the
    matmul's weight read)."""
    f = nc.m.functions[0]
    moved = 0
    for blk in f.blocks:
        out = []
        for inst in blk.instructions:
            si = getattr(inst, "sync_info", None)
            if si is not None and len(si.on_wait) > 1:
                for j, w in enumerate(si.on_wait):
                    nop = mybir.InstNoOp(name=f"{inst.name}-wf{j}", ins=[], outs=[])
                    nop.engine = inst.engine
                    nop.sync_info = mybir.SyncInfo(on_wait=[w], on_update=[])
                    out.append(nop)
                inst.sync_info = mybir.SyncInfo(on_wait=[], on_update=list(si.on_update))
                moved += 1
            out.append(inst)
        blk.instructions = out
    return moved


_NC_CACHE = {}


def get_nc(n_examples=PER_CORE):
    if n_examples not in _NC_CACHE:
        _NC_CACHE[n_examples] = build_nc(n_examples)
    return _NC_CACHE[n_examples]


def make_in_maps(E, S, em, sm, W32):
    """Build the 8 per-core input dicts from full-batch numpy inputs."""
    wt = np.ascontiguousarray(W32.T)
    in_maps = []
    for c in range(NCORES):
        bs = list(range(PER_CORE * c, PER_CORE * (c + 1)))
        et = np.stack([np.ascontiguousarray(E[b].T) for b in bs])
        srct = np.stack([np.ascontiguousarray(S[b].T) for b in bs])
        # srcn rows for masked (padding) src positions are zeroed so the
        # device ctx matmul can consume the UNMASKED exp weights: masked
        # columns contribute exactly zero through the zeroed operand instead
        # of through zeroed weights.
        srcn = np.ascontiguousarray(
            S[bs] * (1 - sm[bs])[:, :, None].astype(np.float32)
        ).astype(NPBF16)
        en = np.ascontiguousarray(E[bs]).astype(NPBF16)
        emadd = np.where(em[bs] != 0, np.float32(NEG), np.float32(0.0)).astype(np.float32)
        emcol = np.ascontiguousarray(emadd.reshape(PER_CORE, KB, P).transpose(0, 2, 1))
        s01r = np.ascontiguousarray((1 - sm[bs]).astype(NPBF16))
        in_maps.append(
            {
                "et": et,
                "srct": srct,
                "srcn": srcn,
                "en": en,
                "wt": wt,
                "emcol": emcol,
                "s01r": s01r,
            }
        )
    return in_maps


def kernel(edit_encodings, src_encodings, edit_sent_masks, src_sent_masks, W):
    E = np.ascontiguousarray(np.asarray(edit_encodings, dtype=np.float32))
    S = np.ascontiguousarray(np.asarray(src_encodings, dtype=np.float32))
    em = np.asarray(edit_sent_masks).astype(np.int32)
    sm = np.asarray(src_sent_masks).astype(np.int32)
    W32 = np.ascontiguousarray(np.asarray(W, dtype=np.float32))

    nc = get_nc()
    in_maps = make_in_maps(E, S, em, sm, W32)
    res = run_bass_kernel_spmd(nc, in_maps, core_ids=list(range(NCORES)))

    edit_ctx = np.empty((B, L, L), np.float32)
    src_ctx = np.empty((B, L, L), np.float32)
    for c in range(NCORES):
        edit_ctx[PER_CORE * c : PER_CORE * (c + 1)] = np.asarray(
            res.results[c]["oe"]
        ).astype(np.float32)
        src_ctx[PER_CORE * c : PER_CORE * (c + 1)] = np.asarray(
            res.results[c]["osr"]
        ).astype(np.float32)
    return edit_ctx, src_ctx

